# revision 46
# baseline (speedup 1.0000x reference)
"""Trainium2 Bass kernel for nn_MessageAggregator (gnn_message_passing). v8

Computation (reference):
    s   = logsig(logsig(state @ W1_m.T + b1_m) @ W2_m.T)      # [E, D]
    agg = mask_transpose @ (mask @ s) - s                     # [E, D]
    out = logsig(logsig([agg, feature] @ W1_a.T + b1_a) @ W2_a.T)

Sharding: edge dimension E=32768 split across 8 cores (4096 edges each).

Pipeline facts this build is shaped around (measured on this stack):
- the first collective cannot complete before ~60us (global barrier) and
  has a ~26us minimum duration; later ops cost ~10us each regardless of
  64KB vs 128KB.  So: ONE fp8 AllReduce of all of v/2 (256KB), triggered
  well before the barrier exits, landing ~87us.
- the PE issues one 512-col matmul per ~216ns; DoubleRow halves the
  instruction count for contraction, not the issue rate.
- phases 0/1 and the mask prefetch all fit in the pre-barrier window;
  the z1a terms that don't depend on the AllReduce (w1anTn@u2T +
  wa2T@featT) are precomputed into zp during the AllReduce wait.
- trainium fp8e4 saturates at 240 and |v| reaches ~250, so the wire and
  vT carry v/2; the x2 is folded into w1anT2.
Phase 2: fp8 DoubleRow mesh against the node-paired mask layout, then a
table-free MLP tail with a single feature-major matmul per output tile.
"""

import ml_dtypes
import numpy as np

N_CORES = 8
E, N, D, DF = 32768, 2048, 128, 32
EL = E // N_CORES          # 4096 edges per core
NT = EL // 128             # 32 edge tiles of 128
NPAIR = NT // 2            # 16 DoubleRow edge pair-tiles
NCH = EL // 512            # 8 output chunks of 512 edges
NB = 8                     # node pair-chunks (256 nodes each) for phase-2 DR
P = 128

AR_FP8 = False             # AllReduce wire dtype fp8e4 (else bf16)
# fp8 wire RDH rounding alone costs ~2e-2 rel err (deterministic, measured)
# -- the wire must be bf16; split in 2 chunks so chunk 0 (which pays the
# ~26us first-collective floor) overlaps chunk 1's wire time.

_CACHE: dict = {}


def _build():
    from concourse import bacc, mybir, tile

    F32 = mybir.dt.float32
    BF16 = mybir.dt.bfloat16
    FP16 = mybir.dt.float16
    FP8 = mybir.dt.float8e4
    AF = mybir.ActivationFunctionType
    ALU = mybir.AluOpType
    DR = mybir.MatmulPerfMode.DoubleRow
    AR_DT = FP8 if AR_FP8 else BF16

    nc = bacc.Bacc("TRN2", target_bir_lowering=False, debug=False,
                   num_devices=N_CORES)

    stateT_l = nc.dram_tensor("stateT_l", [D, EL], FP8, kind="ExternalInput")
    featT_l = nc.dram_tensor("featT_l", [DF, EL], FP8, kind="ExternalInput")
    # mT tiles: [pr, h, p, slot, x] with edge = pr*256+slot*128+p,
    # node = h*1024 + x  (contiguous 256KB per (pr,h) DMA)
    mtp_l = nc.dram_tensor("mtp_l", [NPAIR, 2, P, 2, 1024], FP8,
                           kind="ExternalInput")
    # mask tiles: [b, h, p, slot, x] with node = b*256+slot*128+p,
    # edge = h*1024 + x  (contiguous 256KB per (b,h) DMA)
    mask_l = nc.dram_tensor("mask_l", [NB, 4, P, 2, 1024], FP8,
                            kind="ExternalInput")
    w1mT = nc.dram_tensor("w1mT", [D, D], BF16, kind="ExternalInput")
    w2mnT = nc.dram_tensor("w2mnT", [D, D], BF16, kind="ExternalInput")
    w1anT2 = nc.dram_tensor("w1anT2", [D, D], BF16, kind="ExternalInput")
    w1anTn = nc.dram_tensor("w1anTn", [D, D], BF16, kind="ExternalInput")
    wa2T = nc.dram_tensor("wa2T", [DF, D], BF16, kind="ExternalInput")
    w2anT = nc.dram_tensor("w2anT", [D, D], FP16, kind="ExternalInput")
    nb1m = nc.dram_tensor("nb1m", [D], F32, kind="ExternalInput")
    nb1a = nc.dram_tensor("nb1a", [D], F32, kind="ExternalInput")
    idn_b = nc.dram_tensor("idn_b", [P, P], BF16, kind="ExternalInput")
    # output feature-major [j, d, 512 edges]; host transposes
    out_l = nc.dram_tensor("out_l", [NCH, D, 512], FP16, kind="ExternalOutput")

    with tile.TileContext(nc) as tc:
        with (
            tc.tile_pool(name="consts", bufs=1) as consts,
            tc.tile_pool(name="persist", bufs=1) as persist,
            tc.tile_pool(name="mlp", bufs=4) as mlp,
            tc.tile_pool(name="mlp8", bufs=8) as mlp8,
            tc.tile_pool(name="mtp", bufs=1) as mtp,
            tc.tile_pool(name="maskp", bufs=1) as maskp,
            tc.tile_pool(name="outp", bufs=2) as outp,
            tc.tile_pool(name="ps_acc", bufs=1, space="PSUM") as ps_acc,
            tc.tile_pool(name="ps_mm", bufs=2, space="PSUM") as ps_mm,
            tc.tile_pool(name="ps_tp", bufs=2, space="PSUM") as ps_tp,
            tc.tile_pool(name="dram", bufs=1, space="DRAM") as dram,
        ):
            # ---------------- constants + state (host-prepped) ----------
            w1mT_sb = consts.tile([D, D], BF16)
            nc.sync.dma_start(w1mT_sb[:], w1mT[:])
            w2mnT_sb = consts.tile([D, D], BF16)
            nc.sync.dma_start(w2mnT_sb[:], w2mnT[:])
            w1anT2_sb = consts.tile([D, D], BF16)
            nc.sync.dma_start(w1anT2_sb[:], w1anT2[:])
            w1anTn_sb = consts.tile([D, D], BF16)
            nc.sync.dma_start(w1anTn_sb[:], w1anTn[:])
            wa2T_sb = consts.tile([DF, D], BF16)
            nc.sync.dma_start(wa2T_sb[:], wa2T[:])
            w2anT_sb = consts.tile([D, D], FP16)
            nc.sync.dma_start(w2anT_sb[:], w2anT[:])
            nb1m_sb = consts.tile([D, 1], F32)
            nc.sync.dma_start(nb1m_sb[:], nb1m[:, None])
            nb1a_sb = consts.tile([D, 1], F32)
            nc.sync.dma_start(nb1a_sb[:], nb1a[:, None])
            idn_bf = consts.tile([P, P], BF16)
            nc.sync.dma_start(idn_bf[:], idn_b[:])

            stateT_sb = persist.tile([P, EL], FP8)
            for q8 in range(8):
                nc.sync.dma_start(
                    stateT_sb[:, q8 * 512 : (q8 + 1) * 512],
                    stateT_l[:, q8 * 512 : (q8 + 1) * 512],
                )

            # mT stream: 256KB chunks, pr-major
            mts = {}
            for pr in range(NPAIR):
                mt = mtp.tile([P, 2, N], FP8, tag=f"mt{pr}",
                              name=f"mt_{pr}")
                for h in range(2):
                    nc.sync.dma_start(
                        mt[:, :, h * 1024 : (h + 1) * 1024],
                        mtp_l[pr, h],
                    )
                mts[pr] = mt

            featT = persist.tile([DF, EL], FP8)
            nc.sync.dma_start(featT[:], featT_l[:])

            # ---------------- phase 0: memory MLP (exact Exp+Ln) ---------
            # Single wave of 8 j-tiles per stage; the ACT engine executes
            # its stream in program order so each table loads exactly once
            # (Exp, Ln, Exp, Ln = 4 loads).  ex1/ex2 share one ring.
            u2T = persist.tile([P, EL], BF16)      # -s.T (feat-major)
            u2e = persist.tile([P, NT, D], FP8)    # -s    (edge-major tiles)

            h1s, ex1s, u1s, z2s, ex2s = {}, {}, {}, {}, {}
            for j in range(8):
                h1 = ps_mm.tile([P, 512], F32, tag="mm", name=f"h1_{j}")
                nc.tensor.matmul(
                    h1[:], w1mT_sb[:],
                    stateT_sb[:, j * 512 : (j + 1) * 512],
                    start=True, stop=True,
                )
                h1s[j] = h1
            for j in range(8):
                ex1 = mlp8.tile([P, 512], BF16, tag="exx", name=f"ex1_{j}")
                nc.scalar.activation(ex1[:], h1s[j][:], AF.Exp,
                                     scale=-1.0, bias=nb1m_sb[:])
                ex1s[j] = ex1
            for j in range(8):
                u1 = mlp8.tile([P, 512], BF16, tag="u1", name=f"u1_{j}")
                nc.scalar.activation(u1[:], ex1s[j][:], AF.Ln, bias=1.0)
                u1s[j] = u1
            for j in range(8):
                z2 = ps_mm.tile([P, 512], F32, tag="mm", name=f"z2_{j}")
                nc.tensor.matmul(z2[:], w2mnT_sb[:], u1s[j][:],
                                 start=True, stop=True)
                z2s[j] = z2
            for j in range(8):
                ex2 = mlp8.tile([P, 512], BF16, tag="exx", name=f"ex2_{j}")
                nc.scalar.activation(ex2[:], z2s[j][:], AF.Exp, scale=-1.0)
                ex2s[j] = ex2
            for j in range(8):
                nc.scalar.activation(
                    u2T[:, j * 512 : (j + 1) * 512], ex2s[j][:],
                    AF.Ln, bias=1.0,
                )
            # ---------- phase-0 tail interleaved with phase 1 ------------
            # transposes for edge-pair j feed the phase-1 DR matmuls for
            # pr = 2j, 2j+1 immediately; acc[q] accumulation groups span
            # the whole j loop (start at j=0, stop at j=7).
            # acc[q] = -(mask @ s).T chunk [D, 512 nodes]; wire carries v/2
            # chunk A = nodes 0-1023: only q0/q1 run inside the j-loop so
            # the first collective (which pays the ~26-31us floor) triggers
            # ~10us earlier; q2/q3 run as a second DR pass feeding chunk B,
            # whose wire time hides behind chunk A's op.
            vsb = persist.tile([P, N], AR_DT)      # -v/2 partial, f-major
            cc_ins = [dram.tile([P, 1024], AR_DT, name=f"cc_in{h}")
                      for h in range(2)]
            cc_outs = [dram.tile([P, 1024], AR_DT, addr_space="Shared",
                                 name=f"cc_out{h}")
                       for h in range(2)]
            accs1 = [ps_acc.tile([P, 512], F32, tag=f"acc{q}",
                                 name=f"p1acc{q}") for q in range(4)]
            for j in range(8):
                tp2 = ps_tp.tile([P, 512], BF16, tag="tp", name=f"tp2_{j}")
                for k in range(4):
                    c0 = (j * 4 + k) * P
                    nc.tensor.transpose(
                        tp2[:, k * P : (k + 1) * P],
                        u2T[:, c0 : c0 + P],
                        idn_bf[:],
                    )
                nc.vector.tensor_copy(
                    u2e[:, j * 4 : (j + 1) * 4, :].rearrange(
                        "p a d -> p (a d)"
                    ),
                    tp2[:],
                )
                for pr in (2 * j, 2 * j + 1):
                    for q in range(2):
                        nc.tensor.matmul(
                            accs1[q][:],
                            u2e[:, 2 * pr : 2 * pr + 2, :],
                            mts[pr][:, :, q * 512 : (q + 1) * 512],
                            start=(pr == 0),
                            stop=(pr == NPAIR - 1),
                            perf_mode=DR,
                        )
            for q in range(2):
                nc.vector.tensor_scalar(
                    vsb[:, q * 512 : (q + 1) * 512], accs1[q][:], 0.5,
                    None, ALU.mult,
                )
                # parallel sync rings (mT stream has drained by now)
                nc.sync.dma_start(
                    cc_ins[0][:, q * 512 : (q + 1) * 512],
                    vsb[:, q * 512 : (q + 1) * 512],
                )
            nc.gpsimd.collective_compute(
                "AllReduce",
                mybir.AluOpType.add,
                ins=[cc_ins[0].opt()],
                outs=[cc_outs[0].opt()],
                replica_groups=[list(range(N_CORES))],
            )
            for pr in range(NPAIR):
                for q in (2, 3):
                    nc.tensor.matmul(
                        accs1[q][:],
                        u2e[:, 2 * pr : 2 * pr + 2, :],
                        mts[pr][:, :, q * 512 : (q + 1) * 512],
                        start=(pr == 0),
                        stop=(pr == NPAIR - 1),
                        perf_mode=DR,
                    )
            for q in (2, 3):
                nc.vector.tensor_scalar(
                    vsb[:, q * 512 : (q + 1) * 512], accs1[q][:], 0.5,
                    None, ALU.mult,
                )
                nc.sync.dma_start(
                    cc_ins[1][:, (q - 2) * 512 : (q - 1) * 512],
                    vsb[:, q * 512 : (q + 1) * 512],
                )
            nc.gpsimd.collective_compute(
                "AllReduce",
                mybir.AluOpType.add,
                ins=[cc_ins[1].opt()],
                outs=[cc_outs[1].opt()],
                replica_groups=[list(range(N_CORES))],
            )

            # Scheduler-only fence: the mask DMAs emitted next land behind
            # the mT/state stream in every DMA ring FIFO.
            tc.no_sync_barrier()
            mks = []
            for b in range(NB):
                mk = maskp.tile([P, 2, EL], FP8, tag=f"mk{b}",
                                name=f"mk_{b}")
                for h in range(4):
                    nc.sync.dma_start(
                        mk[:, :, h * 1024 : (h + 1) * 1024],
                        mask_l[b, h],
                    )
                mks.append(mk)

            # -------- zp precompute (AllReduce-independent z1a terms) ----
            # zp = w1anTn.T @ u2T + wa2T.T @ featT, done during the
            # AllReduce wait; folded into z1a via an identity matmul.
            zp_sb = persist.tile([P, EL], BF16)
            for j in range(8):
                zpp = ps_mm.tile([P, 512], F32, tag="mm", name=f"zpp_{j}")
                nc.tensor.matmul(zpp[:], w1anTn_sb[:],
                                 u2T[:, j * 512 : (j + 1) * 512],
                                 start=True, stop=False)
                nc.tensor.matmul(
                    zpp[:], wa2T_sb[:], featT[:, j * 512 : (j + 1) * 512],
                    start=False, stop=True,
                )
                nc.vector.tensor_copy(
                    zp_sb[:, j * 512 : (j + 1) * 512], zpp[:]
                )

            # ---------------- AllReduce receive: vT (node-major fp8) -----
            # pipelined per 512-node chunk: DMA -> transpose -> cast to vT
            vT = persist.tile([P, N // P, D], FP8)   # -v/2  [p, nb, D]
            for q in range(4):
                vfull = mlp.tile([P, 512], AR_DT, tag="vfull",
                                 name=f"vfull{q}")
                src_cc = cc_outs[q // 2][:, (q % 2) * 512 : (q % 2 + 1) * 512]
                nc.sync.dma_start(vfull[:], src_cc)
                tp3 = ps_tp.tile([P, 512], BF16, tag="tp", name=f"tp3_{q}")
                for k in range(4):
                    nc.tensor.transpose(
                        tp3[:, k * P : (k + 1) * P],
                        vfull[:, k * P : (k + 1) * P],
                        idn_bf[:],
                    )
                nc.vector.tensor_copy(
                    vT[:, 4 * q : 4 * q + 4, :].rearrange("p a d -> p (a d)"),
                    tp3[:],
                )

            # ---------------- phase 2: edge agg (DR) + concat MLP --------
            def p2_mesh(js):
                accs = {}
                for j in js:
                    accs[j] = ps_acc.tile([P, 512], F32, tag=f"acc{j % 4}",
                                          name=f"p2acc_{j}")
                for b in range(NB):
                    for j in js:
                        nc.tensor.matmul(
                            accs[j][:],
                            vT[:, 2 * b : 2 * b + 2, :],
                            mks[b][:, :, j * 512 : (j + 1) * 512],
                            start=(b == 0),
                            stop=(b == NB - 1),
                            perf_mode=DR,
                        )
                return accs

            def p2_w3(jacc):
                # acc holds (maskT@(-v))/2; the x2 and the -u2T/feat terms
                # are folded into the z1a accumulation (w1anT2, zp)
                w3s = {}
                for j, acc in jacc:
                    w3 = mlp.tile([P, 512], BF16, tag="w3", name=f"w3_{j}")
                    nc.vector.tensor_copy(w3[:], acc[:])
                    w3s[j] = w3
                return w3s

            def p2_mlp(js, w3s):
                # table-free tail: u3 = relu(-z1a - b1a), out = min(po, 0)
                z1as, u3s = {}, {}
                for j in js:
                    z1a = ps_mm.tile([P, 512], F32, tag="mm", name=f"z1a_{j}")
                    nc.tensor.matmul(z1a[:], w1anT2_sb[:], w3s[j][:],
                                     start=True, stop=False)
                    nc.tensor.matmul(
                        z1a[:], idn_bf[:],
                        zp_sb[:, j * 512 : (j + 1) * 512],
                        start=False, stop=True,
                    )
                    z1as[j] = z1a
                for j in js:
                    u3 = mlp.tile([P, 512], FP16, tag="u3", name=f"u3_{j}")
                    nc.scalar.activation(u3[:], z1as[j][:], AF.Relu,
                                         scale=-1.0, bias=nb1a_sb[:])
                    u3s[j] = u3
                for j in js:
                    po = ps_tp.tile([P, 512], F32, tag="tp", name=f"po_{j}")
                    nc.tensor.matmul(po[:], w2anT_sb[:], u3s[j][:],
                                     start=True, stop=True)
                    ob = outp.tile([P, 512], FP16, tag="ob", name=f"ob_{j}")
                    nc.vector.tensor_scalar(
                        ob[:], po[:], 0.0, None, ALU.min
                    )
                    nc.sync.dma_start(out_l[j, :, :], ob[:])

            js0 = [0, 1, 2, 3]
            js1 = [4, 5, 6, 7]
            acc0 = p2_mesh(js0)
            w3s0 = p2_w3([(j, acc0[j]) for j in js0])
            acc1 = p2_mesh(js1)
            p2_mlp(js0, w3s0)
            w3s1 = p2_w3([(j, acc1[j]) for j in js1])
            p2_mlp(js1, w3s1)
    nc.compile()
    return nc


def kernel(**inputs: np.ndarray) -> np.ndarray:
    from concourse.bass_utils import run_bass_kernel_spmd

    if "nc" not in _CACHE:
        _CACHE["nc"] = _build()
    nc = _CACHE["nc"]

    state = np.ascontiguousarray(inputs["state"], dtype=np.float32)
    feature = np.ascontiguousarray(inputs["feature"], dtype=np.float32)
    mask = np.ascontiguousarray(inputs["mask"], dtype=np.float32)
    mask_transpose = np.ascontiguousarray(
        inputs["mask_transpose"], dtype=np.float32
    )

    W1m = np.asarray(inputs["W1_m"], dtype=np.float32)
    W2m = np.asarray(inputs["W2_m"], dtype=np.float32)
    W1a = np.asarray(inputs["W1_a"], dtype=np.float32)
    W2a = np.asarray(inputs["W2_a"], dtype=np.float32)
    common = {
        "w1mT": np.ascontiguousarray(W1m.T).astype(ml_dtypes.bfloat16),
        "w2mnT": np.ascontiguousarray(-W2m.T).astype(ml_dtypes.bfloat16),
        "w1anT2": np.ascontiguousarray(-2.0 * W1a[:, :D].T).astype(
            ml_dtypes.bfloat16
        ),
        "w1anTn": np.ascontiguousarray(W1a[:, :D].T).astype(
            ml_dtypes.bfloat16
        ),
        "wa2T": np.ascontiguousarray(W1a[:, D:].T).astype(ml_dtypes.bfloat16),
        "w2anT": np.ascontiguousarray(-W2a.T).astype(np.float16),
        "nb1m": -np.asarray(inputs["b1_m"], dtype=np.float32),
        "nb1a": -np.asarray(inputs["b1_a"], dtype=np.float32),
        "idn_b": np.eye(P, dtype=np.float32).astype(ml_dtypes.bfloat16),
    }
    in_maps = []
    for c in range(N_CORES):
        sl = slice(c * EL, (c + 1) * EL)
        # [pr, h, p, slot, x]: edge = pr*256+slot*128+p, node = h*1024+x
        mtp_h = (
            mask_transpose[sl]
            .reshape(NPAIR, 2, P, 2, 1024)
            .transpose(0, 3, 2, 1, 4)
        )
        # [b, h, p, slot, x]: node = b*256+slot*128+p, edge = h*1024+x
        mask_h = (
            mask[:, sl]
            .reshape(NB, 2, P, 4, 1024)
            .transpose(0, 3, 2, 1, 4)
        )
        in_maps.append(
            {
                "stateT_l": np.ascontiguousarray(state[sl].T).astype(
                    ml_dtypes.float8_e4m3fn
                ),
                "featT_l": np.ascontiguousarray(feature[sl].T).astype(
                    ml_dtypes.float8_e4m3fn
                ),
                "mtp_l": np.ascontiguousarray(mtp_h).astype(
                    ml_dtypes.float8_e4m3fn
                ),
                "mask_l": np.ascontiguousarray(mask_h).astype(
                    ml_dtypes.float8_e4m3fn
                ),
                **common,
            }
        )
    _CACHE["in_maps"] = in_maps

    res = run_bass_kernel_spmd(nc, in_maps, core_ids=list(range(N_CORES)))
    outs = []
    for c in range(N_CORES):
        o = np.asarray(res.results[c]["out_l"]).astype(np.float32)
        # [j, d, 512 e] feature-major -> [EL, D]
        o = o.transpose(0, 2, 1).reshape(EL, D)
        outs.append(o)
    return np.concatenate(outs, axis=0)


# revision 47
# speedup vs baseline: 1.0025x; 1.0025x over previous
"""Trainium2 Bass kernel for nn_MessageAggregator (gnn_message_passing). v8

Computation (reference):
    s   = logsig(logsig(state @ W1_m.T + b1_m) @ W2_m.T)      # [E, D]
    agg = mask_transpose @ (mask @ s) - s                     # [E, D]
    out = logsig(logsig([agg, feature] @ W1_a.T + b1_a) @ W2_a.T)

Sharding: edge dimension E=32768 split across 8 cores (4096 edges each).

Pipeline facts this build is shaped around (measured on this stack):
- the first collective cannot complete before ~60us (global barrier) and
  has a ~26us minimum duration; later ops cost ~10us each regardless of
  64KB vs 128KB.  So: ONE fp8 AllReduce of all of v/2 (256KB), triggered
  well before the barrier exits, landing ~87us.
- the PE issues one 512-col matmul per ~216ns; DoubleRow halves the
  instruction count for contraction, not the issue rate.
- phases 0/1 and the mask prefetch all fit in the pre-barrier window;
  the z1a terms that don't depend on the AllReduce (w1anTn@u2T +
  wa2T@featT) are precomputed into zp during the AllReduce wait.
- trainium fp8e4 saturates at 240 and |v| reaches ~250, so the wire and
  vT carry v/2; the x2 is folded into w1anT2.
Phase 2: fp8 DoubleRow mesh against the node-paired mask layout, then a
table-free MLP tail with a single feature-major matmul per output tile.
"""

import ml_dtypes
import numpy as np

N_CORES = 8
E, N, D, DF = 32768, 2048, 128, 32
EL = E // N_CORES          # 4096 edges per core
NT = EL // 128             # 32 edge tiles of 128
NPAIR = NT // 2            # 16 DoubleRow edge pair-tiles
NCH = EL // 512            # 8 output chunks of 512 edges
NB = 8                     # node pair-chunks (256 nodes each) for phase-2 DR
P = 128

AR_FP8 = False             # AllReduce wire dtype fp8e4 (else bf16)
# fp8 wire RDH rounding alone costs ~2e-2 rel err (deterministic, measured)
# -- the wire must be bf16; split in 2 chunks so chunk 0 (which pays the
# ~26us first-collective floor) overlaps chunk 1's wire time.

_CACHE: dict = {}


def _build():
    from concourse import bacc, mybir, tile

    F32 = mybir.dt.float32
    BF16 = mybir.dt.bfloat16
    FP16 = mybir.dt.float16
    FP8 = mybir.dt.float8e4
    AF = mybir.ActivationFunctionType
    ALU = mybir.AluOpType
    DR = mybir.MatmulPerfMode.DoubleRow
    AR_DT = FP8 if AR_FP8 else BF16

    nc = bacc.Bacc("TRN2", target_bir_lowering=False, debug=False,
                   num_devices=N_CORES)

    stateT_l = nc.dram_tensor("stateT_l", [D, EL], FP8, kind="ExternalInput")
    featT_l = nc.dram_tensor("featT_l", [DF, EL], FP8, kind="ExternalInput")
    # mT tiles: [pr, h, p, slot, x] with edge = pr*256+slot*128+p,
    # node = h*1024 + x  (contiguous 256KB per (pr,h) DMA)
    mtp_l = nc.dram_tensor("mtp_l", [NPAIR, 2, P, 2, 1024], FP8,
                           kind="ExternalInput")
    # mask tiles: [b, h, p, slot, x] with node = b*256+slot*128+p,
    # edge = h*1024 + x  (contiguous 256KB per (b,h) DMA)
    mask_l = nc.dram_tensor("mask_l", [NB, 4, P, 2, 1024], FP8,
                            kind="ExternalInput")
    w1mT = nc.dram_tensor("w1mT", [D, D], BF16, kind="ExternalInput")
    w2mnT = nc.dram_tensor("w2mnT", [D, D], BF16, kind="ExternalInput")
    w1anT2 = nc.dram_tensor("w1anT2", [D, D], BF16, kind="ExternalInput")
    w1anTn = nc.dram_tensor("w1anTn", [D, D], BF16, kind="ExternalInput")
    wa2T = nc.dram_tensor("wa2T", [DF, D], BF16, kind="ExternalInput")
    w2anT = nc.dram_tensor("w2anT", [D, D], FP16, kind="ExternalInput")
    nb1m = nc.dram_tensor("nb1m", [D], F32, kind="ExternalInput")
    nb1a = nc.dram_tensor("nb1a", [D], F32, kind="ExternalInput")
    idn_b = nc.dram_tensor("idn_b", [P, P], BF16, kind="ExternalInput")
    # output feature-major [j, d, 512 edges]; host transposes
    out_l = nc.dram_tensor("out_l", [NCH, D, 512], FP16, kind="ExternalOutput")

    with tile.TileContext(nc) as tc:
        with (
            tc.tile_pool(name="consts", bufs=1) as consts,
            tc.tile_pool(name="persist", bufs=1) as persist,
            tc.tile_pool(name="mlp", bufs=4) as mlp,
            tc.tile_pool(name="mlp8", bufs=8) as mlp8,
            tc.tile_pool(name="mtp", bufs=1) as mtp,
            tc.tile_pool(name="maskp", bufs=1) as maskp,
            tc.tile_pool(name="outp", bufs=2) as outp,
            tc.tile_pool(name="ps_acc", bufs=1, space="PSUM") as ps_acc,
            tc.tile_pool(name="ps_mm", bufs=2, space="PSUM") as ps_mm,
            tc.tile_pool(name="ps_tp", bufs=2, space="PSUM") as ps_tp,
            tc.tile_pool(name="dram", bufs=1, space="DRAM") as dram,
        ):
            # ---------------- constants + state (host-prepped) ----------
            w1mT_sb = consts.tile([D, D], BF16)
            nc.sync.dma_start(w1mT_sb[:], w1mT[:])
            w2mnT_sb = consts.tile([D, D], BF16)
            nc.sync.dma_start(w2mnT_sb[:], w2mnT[:])
            w1anT2_sb = consts.tile([D, D], BF16)
            nc.sync.dma_start(w1anT2_sb[:], w1anT2[:])
            w1anTn_sb = consts.tile([D, D], BF16)
            nc.sync.dma_start(w1anTn_sb[:], w1anTn[:])
            wa2T_sb = consts.tile([DF, D], BF16)
            nc.sync.dma_start(wa2T_sb[:], wa2T[:])
            w2anT_sb = consts.tile([D, D], FP16)
            nc.sync.dma_start(w2anT_sb[:], w2anT[:])
            nb1m_sb = consts.tile([D, 1], F32)
            nc.sync.dma_start(nb1m_sb[:], nb1m[:, None])
            nb1a_sb = consts.tile([D, 1], F32)
            nc.sync.dma_start(nb1a_sb[:], nb1a[:, None])
            idn_bf = consts.tile([P, P], BF16)
            nc.sync.dma_start(idn_bf[:], idn_b[:])

            stateT_sb = persist.tile([P, EL], FP8)
            for q8 in range(8):
                nc.sync.dma_start(
                    stateT_sb[:, q8 * 512 : (q8 + 1) * 512],
                    stateT_l[:, q8 * 512 : (q8 + 1) * 512],
                )

            # mT stream: 256KB chunks, pr-major
            mts = {}
            for pr in range(NPAIR):
                mt = mtp.tile([P, 2, N], FP8, tag=f"mt{pr}",
                              name=f"mt_{pr}")
                for h in range(2):
                    nc.sync.dma_start(
                        mt[:, :, h * 1024 : (h + 1) * 1024],
                        mtp_l[pr, h],
                    )
                mts[pr] = mt

            featT = persist.tile([DF, EL], FP8)
            nc.sync.dma_start(featT[:], featT_l[:])

            # ---------------- phase 0: memory MLP (exact Exp+Ln) ---------
            # Single wave of 8 j-tiles per stage; the ACT engine executes
            # its stream in program order so each table loads exactly once
            # (Exp, Ln, Exp, Ln = 4 loads).  ex1/ex2 share one ring.
            u2T = persist.tile([P, EL], BF16)      # -s.T (feat-major)
            u2e = persist.tile([P, NT, D], FP8)    # -s    (edge-major tiles)

            h1s, ex1s, u1s, z2s, ex2s = {}, {}, {}, {}, {}
            for j in range(8):
                h1 = ps_mm.tile([P, 512], F32, tag="mm", name=f"h1_{j}")
                nc.tensor.matmul(
                    h1[:], w1mT_sb[:],
                    stateT_sb[:, j * 512 : (j + 1) * 512],
                    start=True, stop=True,
                )
                h1s[j] = h1
            for j in range(8):
                ex1 = mlp8.tile([P, 512], BF16, tag="exx", name=f"ex1_{j}")
                nc.scalar.activation(ex1[:], h1s[j][:], AF.Exp,
                                     scale=-1.0, bias=nb1m_sb[:])
                ex1s[j] = ex1
            for j in range(8):
                u1 = mlp8.tile([P, 512], BF16, tag="u1", name=f"u1_{j}")
                nc.scalar.activation(u1[:], ex1s[j][:], AF.Ln, bias=1.0)
                u1s[j] = u1
            for j in range(8):
                z2 = ps_mm.tile([P, 512], F32, tag="mm", name=f"z2_{j}")
                nc.tensor.matmul(z2[:], w2mnT_sb[:], u1s[j][:],
                                 start=True, stop=True)
                z2s[j] = z2
            for j in range(8):
                ex2 = mlp8.tile([P, 512], BF16, tag="exx", name=f"ex2_{j}")
                nc.scalar.activation(ex2[:], z2s[j][:], AF.Exp, scale=-1.0)
                ex2s[j] = ex2
            for j in range(8):
                nc.scalar.activation(
                    u2T[:, j * 512 : (j + 1) * 512], ex2s[j][:],
                    AF.Ln, bias=1.0,
                )
            # ---------- phase-0 tail interleaved with phase 1 ------------
            # transposes for edge-pair j feed the phase-1 DR matmuls for
            # pr = 2j, 2j+1 immediately; acc[q] accumulation groups span
            # the whole j loop (start at j=0, stop at j=7).
            # acc[q] = -(mask @ s).T chunk [D, 512 nodes]; wire carries v/2
            # chunk 0 = nodes 0-1535 (384KB ~= the first-collective floor,
            # triggered before the q=3 matmuls run); chunk 1 = nodes
            # 1536-2047 (128KB, ~11us op).
            vsb = persist.tile([P, N], AR_DT)      # -v/2 partial, f-major
            AR_COL = [(0, 1536), (1536, 2048)]
            cc_ins = [dram.tile([P, c1 - c0], AR_DT, name=f"cc_in{h}")
                      for h, (c0, c1) in enumerate(AR_COL)]
            cc_outs = [dram.tile([P, c1 - c0], AR_DT, addr_space="Shared",
                                 name=f"cc_out{h}")
                       for h, (c0, c1) in enumerate(AR_COL)]
            accs1 = [ps_acc.tile([P, 512], F32, tag=f"acc{q}",
                                 name=f"p1acc{q}") for q in range(4)]
            for j in range(8):
                tp2 = ps_tp.tile([P, 512], BF16, tag="tp", name=f"tp2_{j}")
                for k in range(4):
                    c0 = (j * 4 + k) * P
                    nc.tensor.transpose(
                        tp2[:, k * P : (k + 1) * P],
                        u2T[:, c0 : c0 + P],
                        idn_bf[:],
                    )
                nc.vector.tensor_copy(
                    u2e[:, j * 4 : (j + 1) * 4, :].rearrange(
                        "p a d -> p (a d)"
                    ),
                    tp2[:],
                )
                for pr in (2 * j, 2 * j + 1):
                    for q in range(3):
                        nc.tensor.matmul(
                            accs1[q][:],
                            u2e[:, 2 * pr : 2 * pr + 2, :],
                            mts[pr][:, :, q * 512 : (q + 1) * 512],
                            start=(pr == 0),
                            stop=(pr == NPAIR - 1),
                            perf_mode=DR,
                        )
            for q in range(3):
                nc.vector.tensor_scalar(
                    vsb[:, q * 512 : (q + 1) * 512], accs1[q][:], 0.5,
                    None, ALU.mult,
                )
                # parallel sync rings (mT stream has drained by now)
                nc.sync.dma_start(
                    cc_ins[0][:, q * 512 : (q + 1) * 512],
                    vsb[:, q * 512 : (q + 1) * 512],
                )
            nc.gpsimd.collective_compute(
                "AllReduce",
                mybir.AluOpType.add,
                ins=[cc_ins[0].opt()],
                outs=[cc_outs[0].opt()],
                replica_groups=[list(range(N_CORES))],
            )
            for pr in range(NPAIR):
                nc.tensor.matmul(
                    accs1[3][:],
                    u2e[:, 2 * pr : 2 * pr + 2, :],
                    mts[pr][:, :, 1536:2048],
                    start=(pr == 0),
                    stop=(pr == NPAIR - 1),
                    perf_mode=DR,
                )
            nc.vector.tensor_scalar(
                vsb[:, 1536:2048], accs1[3][:], 0.5, None, ALU.mult
            )
            nc.sync.dma_start(cc_ins[1][:], vsb[:, 1536:2048])
            nc.gpsimd.collective_compute(
                "AllReduce",
                mybir.AluOpType.add,
                ins=[cc_ins[1].opt()],
                outs=[cc_outs[1].opt()],
                replica_groups=[list(range(N_CORES))],
            )

            # Scheduler-only fence: the mask DMAs emitted next land behind
            # the mT/state stream in every DMA ring FIFO.
            tc.no_sync_barrier()
            mks = []
            for b in range(NB):
                mk = maskp.tile([P, 2, EL], FP8, tag=f"mk{b}",
                                name=f"mk_{b}")
                for h in range(4):
                    nc.sync.dma_start(
                        mk[:, :, h * 1024 : (h + 1) * 1024],
                        mask_l[b, h],
                    )
                mks.append(mk)

            # -------- zp precompute (AllReduce-independent z1a terms) ----
            # zp = w1anTn.T @ u2T + wa2T.T @ featT, done during the
            # AllReduce wait; folded into z1a via an identity matmul.
            zp_sb = persist.tile([P, EL], BF16)
            for j in range(8):
                zpp = ps_mm.tile([P, 512], F32, tag="mm", name=f"zpp_{j}")
                nc.tensor.matmul(zpp[:], w1anTn_sb[:],
                                 u2T[:, j * 512 : (j + 1) * 512],
                                 start=True, stop=False)
                nc.tensor.matmul(
                    zpp[:], wa2T_sb[:], featT[:, j * 512 : (j + 1) * 512],
                    start=False, stop=True,
                )
                nc.vector.tensor_copy(
                    zp_sb[:, j * 512 : (j + 1) * 512], zpp[:]
                )

            # ---------------- AllReduce receive: vT (node-major fp8) -----
            # pipelined per 512-node chunk: DMA -> transpose -> cast to vT
            vT = persist.tile([P, N // P, D], FP8)   # -v/2  [p, nb, D]
            for q in range(4):
                vfull = mlp.tile([P, 512], AR_DT, tag="vfull",
                                 name=f"vfull{q}")
                if q < 3:
                    src_cc = cc_outs[0][:, q * 512 : (q + 1) * 512]
                else:
                    src_cc = cc_outs[1][:]
                nc.sync.dma_start(vfull[:], src_cc)
                tp3 = ps_tp.tile([P, 512], BF16, tag="tp", name=f"tp3_{q}")
                for k in range(4):
                    nc.tensor.transpose(
                        tp3[:, k * P : (k + 1) * P],
                        vfull[:, k * P : (k + 1) * P],
                        idn_bf[:],
                    )
                nc.vector.tensor_copy(
                    vT[:, 4 * q : 4 * q + 4, :].rearrange("p a d -> p (a d)"),
                    tp3[:],
                )

            # ---------------- phase 2: edge agg (DR) + concat MLP --------
            def p2_mesh(js):
                accs = {}
                for j in js:
                    accs[j] = ps_acc.tile([P, 512], F32, tag=f"acc{j % 4}",
                                          name=f"p2acc_{j}")
                for b in range(NB):
                    for j in js:
                        nc.tensor.matmul(
                            accs[j][:],
                            vT[:, 2 * b : 2 * b + 2, :],
                            mks[b][:, :, j * 512 : (j + 1) * 512],
                            start=(b == 0),
                            stop=(b == NB - 1),
                            perf_mode=DR,
                        )
                return accs

            def p2_w3(jacc):
                # acc holds (maskT@(-v))/2; the x2 and the -u2T/feat terms
                # are folded into the z1a accumulation (w1anT2, zp)
                w3s = {}
                for j, acc in jacc:
                    w3 = mlp.tile([P, 512], BF16, tag="w3", name=f"w3_{j}")
                    nc.vector.tensor_copy(w3[:], acc[:])
                    w3s[j] = w3
                return w3s

            def p2_mlp(js, w3s):
                # table-free tail: u3 = relu(-z1a - b1a), out = min(po, 0)
                z1as, u3s = {}, {}
                for j in js:
                    z1a = ps_mm.tile([P, 512], F32, tag="mm", name=f"z1a_{j}")
                    nc.tensor.matmul(z1a[:], w1anT2_sb[:], w3s[j][:],
                                     start=True, stop=False)
                    nc.tensor.matmul(
                        z1a[:], idn_bf[:],
                        zp_sb[:, j * 512 : (j + 1) * 512],
                        start=False, stop=True,
                    )
                    z1as[j] = z1a
                for j in js:
                    u3 = mlp.tile([P, 512], FP16, tag="u3", name=f"u3_{j}")
                    nc.scalar.activation(u3[:], z1as[j][:], AF.Relu,
                                         scale=-1.0, bias=nb1a_sb[:])
                    u3s[j] = u3
                for j in js:
                    po = ps_tp.tile([P, 512], F32, tag="tp", name=f"po_{j}")
                    nc.tensor.matmul(po[:], w2anT_sb[:], u3s[j][:],
                                     start=True, stop=True)
                    ob = outp.tile([P, 512], FP16, tag="ob", name=f"ob_{j}")
                    nc.vector.tensor_scalar(
                        ob[:], po[:], 0.0, None, ALU.min
                    )
                    nc.sync.dma_start(out_l[j, :, :], ob[:])

            js0 = [0, 1, 2, 3]
            js1 = [4, 5, 6, 7]
            acc0 = p2_mesh(js0)
            w3s0 = p2_w3([(j, acc0[j]) for j in js0])
            acc1 = p2_mesh(js1)
            p2_mlp(js0, w3s0)
            w3s1 = p2_w3([(j, acc1[j]) for j in js1])
            p2_mlp(js1, w3s1)
    nc.compile()
    return nc


def kernel(**inputs: np.ndarray) -> np.ndarray:
    from concourse.bass_utils import run_bass_kernel_spmd

    if "nc" not in _CACHE:
        _CACHE["nc"] = _build()
    nc = _CACHE["nc"]

    state = np.ascontiguousarray(inputs["state"], dtype=np.float32)
    feature = np.ascontiguousarray(inputs["feature"], dtype=np.float32)
    mask = np.ascontiguousarray(inputs["mask"], dtype=np.float32)
    mask_transpose = np.ascontiguousarray(
        inputs["mask_transpose"], dtype=np.float32
    )

    W1m = np.asarray(inputs["W1_m"], dtype=np.float32)
    W2m = np.asarray(inputs["W2_m"], dtype=np.float32)
    W1a = np.asarray(inputs["W1_a"], dtype=np.float32)
    W2a = np.asarray(inputs["W2_a"], dtype=np.float32)
    common = {
        "w1mT": np.ascontiguousarray(W1m.T).astype(ml_dtypes.bfloat16),
        "w2mnT": np.ascontiguousarray(-W2m.T).astype(ml_dtypes.bfloat16),
        "w1anT2": np.ascontiguousarray(-2.0 * W1a[:, :D].T).astype(
            ml_dtypes.bfloat16
        ),
        "w1anTn": np.ascontiguousarray(W1a[:, :D].T).astype(
            ml_dtypes.bfloat16
        ),
        "wa2T": np.ascontiguousarray(W1a[:, D:].T).astype(ml_dtypes.bfloat16),
        "w2anT": np.ascontiguousarray(-W2a.T).astype(np.float16),
        "nb1m": -np.asarray(inputs["b1_m"], dtype=np.float32),
        "nb1a": -np.asarray(inputs["b1_a"], dtype=np.float32),
        "idn_b": np.eye(P, dtype=np.float32).astype(ml_dtypes.bfloat16),
    }
    in_maps = []
    for c in range(N_CORES):
        sl = slice(c * EL, (c + 1) * EL)
        # [pr, h, p, slot, x]: edge = pr*256+slot*128+p, node = h*1024+x
        mtp_h = (
            mask_transpose[sl]
            .reshape(NPAIR, 2, P, 2, 1024)
            .transpose(0, 3, 2, 1, 4)
        )
        # [b, h, p, slot, x]: node = b*256+slot*128+p, edge = h*1024+x
        mask_h = (
            mask[:, sl]
            .reshape(NB, 2, P, 4, 1024)
            .transpose(0, 3, 2, 1, 4)
        )
        in_maps.append(
            {
                "stateT_l": np.ascontiguousarray(state[sl].T).astype(
                    ml_dtypes.float8_e4m3fn
                ),
                "featT_l": np.ascontiguousarray(feature[sl].T).astype(
                    ml_dtypes.float8_e4m3fn
                ),
                "mtp_l": np.ascontiguousarray(mtp_h).astype(
                    ml_dtypes.float8_e4m3fn
                ),
                "mask_l": np.ascontiguousarray(mask_h).astype(
                    ml_dtypes.float8_e4m3fn
                ),
                **common,
            }
        )
    _CACHE["in_maps"] = in_maps

    res = run_bass_kernel_spmd(nc, in_maps, core_ids=list(range(N_CORES)))
    outs = []
    for c in range(N_CORES):
        o = np.asarray(res.results[c]["out_l"]).astype(np.float32)
        # [j, d, 512 e] feature-major -> [EL, D]
        o = o.transpose(0, 2, 1).reshape(EL, D)
        outs.append(o)
    return np.concatenate(outs, axis=0)
